# revision 10
# baseline (speedup 1.0000x reference)
"""ConvLSTM block kernel for 8 Trainium2 NeuronCores.

Problem: inputs (B=8, S=10, C=7, H=48, W=48); each of the 7 variables runs an
independent ConvLSTM (shared 3x3 conv, 33->128 channels) over S=10 steps.
Outputs: hidden states (B, S, 224, H, W) and raw gates (7, S, B, 128, H, W).

Sharding: 56 independent sequences (variable-major: q = v*B + b) split 7 per
core. Each core runs its 7 ConvLSTMs sequentially over time, pipelining the
7 sequences within each timestep across engines.

Conv-as-matmul: channel-major. Contraction K=100 packs the 3 vertical taps
(ky) of the 33 input channels (x + 32 h) on partitions, plus a ones-row that
folds in the bias; the 3 horizontal taps (kx) are 3 PSUM-accumulating
matmuls whose moving-operand access patterns are column-shifted views of a
zero-padded (100, 48, 50) buffer. Output gate channels are permuted to
[i, f, o, c] so a single 96-partition sigmoid covers i/f/o.
"""

import numpy as np

import concourse.bass as bass
import concourse.bacc as bacc
import concourse.tile as tile
from concourse import mybir
from concourse.bass_utils import run_bass_kernel_spmd

F32 = mybir.dt.float32

N_CORES = 8
SEQS = 7          # sequences per core
S = 10            # timesteps
H = 48
W = 48
NF = 32           # hidden features
NC_IN = 1 + NF    # conv input channels (x + h)
NG = 4 * NF       # gate channels (128)
WP = W + 2        # padded width
PIX = H * W

# gate channel permutation: ours [i, f, o, c] -> original [i, f, c, o]
# ours m in [0,64)   -> orig m        (i, f)
# ours m in [64,96)  -> orig m + 32   (o)
# ours m in [96,128) -> orig m - 32   (c)


def _perm_to_orig(m):
    if m < 64:
        return m
    if m < 96:
        return m + 32
    return m - 32


def make_lhst(Wk: np.ndarray, bk: np.ndarray) -> np.ndarray:
    """Build the 3 stationary operands, shape (3, 100, 128), kx-major.

    lhsT[kx, 33*g + ic, m] = Wk[perm(m), ic, g, kx]   (g = ky)
    lhsT[0, 99, m] = bk[perm(m)]  (bias rides on the kx=0 accumulation)
    """
    lhst = np.zeros((3, 100, 128), dtype=np.float32)
    perm = np.array([_perm_to_orig(m) for m in range(128)])
    # Wk: (128, 33, 3, 3) = (oc, ic, ky, kx)
    wp = Wk[perm]  # (128, 33, 3, 3)
    for kx in range(3):
        for g in range(3):
            # h channels of ky-tap g at partitions 32g..32g+31
            lhst[kx, 32 * g: 32 * g + 32, :] = wp[:, 1:33, g, kx].T
            # x channel of ky-tap g at partition 96+g
            lhst[kx, 96 + g, :] = wp[:, 0, g, kx]
    lhst[0, 99, :] = bk[perm]
    return lhst


def emit(tc, nc, x_in, w_in, ones_in, hy_out, g_out, reps=1, copy_engine="gpsimd"):
    """Emit the per-core program.

    x_in:  (SEQS, S, H, W) dram
    w_in:  (3, 100, 128) dram
    hy_out: (SEQS, S, NF, H, W) dram
    g_out:  (SEQS, S, NG, H, W) dram
    """
    import contextlib
    ctx = contextlib.ExitStack()
    state = ctx.enter_context(tc.tile_pool(name="state", bufs=1))
    consts = ctx.enter_context(tc.tile_pool(name="consts", bufs=1))
    work = ctx.enter_context(tc.tile_pool(name="work", bufs=2))
    psum = ctx.enter_context(tc.tile_pool(name="psum", bufs=2, space="PSUM"))

    # B3: 3 row-shifted copies of the padded conv input, all seqs.
    # partition 32*g + c  (g<3) : h channel c shifted by ky=g
    # partition 96 + g          : x shifted by ky=g
    # partition 99              : ones (bias row)
    # Group g holds P[yy + g] where P is the (50, 50)-padded image
    # (rows 0 and 49 of P are the vertical zero padding).
    b3 = state.tile([100, SEQS, H, WP], F32)
    # cell state, all seqs: allocated on partitions 0..63, used at [32:64]
    # so it shares partitions with the f-gate slice of sif (TT inputs must
    # be on identical partition ranges)
    cst64 = state.tile([2 * NF, SEQS, H, W], F32)

    wt = consts.tile([100, 3, 128], F32)
    nc.sync.dma_start(out=wt[:, :, :], in_=w_in.rearrange("k p m -> p k m"))

    # engine handles ("dma" routes the replication copies through DMA)
    cpeng = nc.gpsimd if copy_engine == "dma" else getattr(nc, copy_engine)

    def init_state():
        nc.vector.memset(b3[:, :, :, :], 0.0)
        nc.gpsimd.memset(cst64[32:64, :, :, :], 0.0)
        for s in range(SEQS):
            nc.sync.dma_start(out=b3[99:100, s, :, :], in_=ones_in[:, :, :])

    # x staging: per (seq, step), write x into the 3 shifted slots.
    # g=0: B3 rows 1..47  <- x rows 0..46
    # g=1: B3 rows 0..47  <- x rows 0..47
    # g=2: B3 rows 0..46  <- x rows 1..47
    def stage_x(s, t):
        nc.sync.dma_start(
            out=b3[96:97, s, 1:48, 1:49], in_=x_in[s: s + 1, t, 0:47, :])
        nc.sync.dma_start(
            out=b3[97:98, s, 0:48, 1:49], in_=x_in[s: s + 1, t, 0:48, :])
        nc.sync.dma_start(
            out=b3[98:99, s, 0:47, 1:49], in_=x_in[s: s + 1, t, 1:48, :])

    for rep in range(reps):
        init_state()
        for s in range(SEQS):
            stage_x(s, 0)

        for t in range(S):
            for s in range(SEQS):
                # ---- conv: 2 half-images x 3 chunks x 3 kx matmuls ----
                # psum half tile: (128, 3 chunks, 512-padded); chunk = 8 rows
                halves = []
                for hf in range(2):
                    ps = psum.tile([128, 3, 512], F32, tag="ps")
                    halves.append(ps)
                    for kx in range(3):
                        for c in range(3):
                            y0 = 24 * hf + 8 * c
                            nc.tensor.matmul(
                                ps[:, c, 0:384],
                                wt[:, kx, :],
                                b3[:, s, y0: y0 + 8, kx: kx + 48],
                                start=(kx == 0),
                                stop=(kx == 2),
                            )

                # ---- epilogue ----
                sif = work.tile([96, 2, 3, 384], F32, tag="sif")
                # t2 on partitions [32:64] (same range as cst/f);
                # cg on [0:32] (same range as the i slice of sif);
                # tcy on [64:96] (same range as the o slice). cg and tcy
                # share t2's slots (equal per-partition bytes, staggered
                # lifetimes).
                t2 = work.tile([2 * NF, 2, 3, 384], F32, tag="t2")
                cg = work.tile([NF, 2, 3, 384], F32, tag="t2")
                gr = work.tile([96, 2, 3, 384], F32, tag="gr")
                for hf in range(2):
                    ps = halves[hf]
                    # evacuate raw gates PSUM -> SBUF (i,f,o into gr;
                    # c shifted down to partitions 0..31 in cg)
                    if hf == 0:
                        nc.scalar.activation(
                            out=gr[:, hf, :, :], in_=ps[0:96, :, 0:384],
                            func=mybir.ActivationFunctionType.Copy)
                        nc.scalar.activation(
                            out=cg[:, hf, :, :], in_=ps[96:128, :, 0:384],
                            func=mybir.ActivationFunctionType.Copy)
                    else:
                        nc.vector.tensor_copy(gr[:, hf, :, :],
                                              ps[0:96, :, 0:384])
                        nc.vector.tensor_copy(cg[:, hf, :, :],
                                              ps[96:128, :, 0:384])
                    # raw gates out (un-permute [i,f,o,c] -> [i,f,c,o])
                    dst = g_out[s, t, :, 24 * hf: 24 * hf + 24, :]
                    dst = dst.rearrange("g (a y) x -> g a (y x)", a=3)
                    nc.sync.dma_start(out=dst[0:64], in_=gr[0:64, hf])
                    nc.sync.dma_start(out=dst[96:128], in_=gr[64:96, hf])
                    nc.sync.dma_start(out=dst[64:96], in_=cg[:, hf])
                    # sigmoid(i, f, o)
                    nc.scalar.activation(
                        out=sif[:, hf, :, :], in_=ps[0:96, :, 0:384],
                        func=mybir.ActivationFunctionType.Sigmoid)
                    # t2 = i * c_gate (both on partitions [0:32)),
                    # result placed on [32:64)
                    nc.vector.tensor_mul(
                        t2[32:64, hf, :, :], sif[0:32, hf, :, :],
                        cg[:, hf, :, :])

                c_s = cst64[32:64, s, :, :]
                sif_flat = sif.rearrange("p a b c -> p (a b c)")
                t2_flat = t2.rearrange("p a b c -> p (a b c)")
                c_flat = c_s.rearrange("p y x -> p (y x)")
                # c = f * c  (in place; f and c both on [32:64))
                if copy_engine == "dma":
                    nc.gpsimd.tensor_mul(c_flat, sif_flat[32:64], c_flat)
                else:
                    nc.vector.tensor_mul(c_flat, sif_flat[32:64], c_flat)
                # c += t2
                nc.vector.tensor_add(c_flat, c_flat, t2_flat[32:64])
                # tcy = tanh(c), placed on partitions [64:96) to pair
                # with the o slice of sif
                # shares the "t2" slots: same per-partition size, disjoint
                # lifetime (t2 is dead after the add above)
                tcy96 = work.tile([3 * NF, H, W], F32, tag="t2")
                tcy = tcy96[64:96]
                nc.scalar.activation(
                    out=tcy[:, :, :], in_=c_s,
                    func=mybir.ActivationFunctionType.Tanh)
                # hy = o * tcy -> write into B3 group 1 interior
                sif_o = sif_flat[64:96].rearrange("p (y x) -> p y x", y=H)
                nc.vector.tensor_mul(
                    b3[32:64, s, 0:48, 1:49], sif_o, tcy[:, :, :])

                # replicate hy into groups 0 and 2 (row-shifted)
                if copy_engine == "dma":
                    nc.sync.dma_start(
                        out=b3[0:32, s, 1:48, 1:49],
                        in_=b3[32:64, s, 0:47, 1:49])
                    nc.sync.dma_start(
                        out=b3[64:96, s, 0:47, 1:49],
                        in_=b3[32:64, s, 1:48, 1:49])
                else:
                    cpeng.tensor_copy(
                        out=b3[0:32, s, 1:48, 1:49],
                        in_=b3[32:64, s, 0:47, 1:49])
                    cpeng.tensor_copy(
                        out=b3[64:96, s, 0:47, 1:49],
                        in_=b3[32:64, s, 1:48, 1:49])

                # hy to DRAM
                nc.sync.dma_start(
                    out=hy_out[s, t], in_=b3[32:64, s, 0:48, 1:49])

                # stage x for next step
                if t + 1 < S:
                    stage_x(s, t + 1)

    ctx.close()


_CACHE = {}


def build_program(reps=1, copy_engine="gpsimd"):
    key = (reps, copy_engine)
    if key in _CACHE:
        return _CACHE[key]
    nc = bacc.Bacc("TRN2", target_bir_lowering=False, debug=False,
                   num_devices=N_CORES)
    x_in = nc.dram_tensor("x", [SEQS, S, H, W], F32, kind="ExternalInput").ap()
    w_in = nc.dram_tensor("w", [3, 100, 128], F32, kind="ExternalInput").ap()
    ones_in = nc.dram_tensor("one", [1, H, WP], F32, kind="ExternalInput").ap()
    hy_out = nc.dram_tensor("hy", [SEQS, S, NF, H, W], F32,
                            kind="ExternalOutput").ap()
    g_out = nc.dram_tensor("g", [SEQS, S, NG, H, W], F32,
                           kind="ExternalOutput").ap()
    with tile.TileContext(nc) as tc:
        emit(tc, nc, x_in, w_in, ones_in, hy_out, g_out, reps=reps,
             copy_engine=copy_engine)
    nc.compile()
    _CACHE[key] = nc
    return nc


def shard_inputs(inputs: np.ndarray):
    """inputs (B, S, C, H, W) -> list of per-core (SEQS, S, H, W) arrays."""
    B = inputs.shape[0]
    shards = []
    for k in range(N_CORES):
        xs = np.empty((SEQS, S, H, W), dtype=np.float32)
        for j in range(SEQS):
            q = SEQS * k + j
            v, b = divmod(q, B)
            xs[j] = inputs[b, :, v]
        shards.append(np.ascontiguousarray(xs))
    return shards


def gather_outputs(results, B):
    out = np.empty((B, S, 7 * NF, H, W), dtype=np.float32)
    gates = np.empty((7, S, B, NG, H, W), dtype=np.float32)
    for k in range(N_CORES):
        hy = results[k]["hy"]      # (SEQS, S, NF, H, W)
        g = results[k]["g"]        # (SEQS, S, NG, H, W)
        for j in range(SEQS):
            q = SEQS * k + j
            v, b = divmod(q, B)
            out[b, :, NF * v: NF * (v + 1)] = hy[j]
            gates[v, :, b] = g[j]
    return out, gates


def kernel(inputs: np.ndarray, Wk: np.ndarray, bk: np.ndarray):
    inputs = np.asarray(inputs, dtype=np.float32)
    Wk = np.asarray(Wk, dtype=np.float32)
    bk = np.asarray(bk, dtype=np.float32)
    B = inputs.shape[0]

    nc = build_program()
    lhst = make_lhst(Wk, bk)
    shards = shard_inputs(inputs)
    ones = np.ones((1, H, WP), dtype=np.float32)
    in_maps = [{"x": shards[k], "w": lhst, "one": ones}
               for k in range(N_CORES)]
    res = run_bass_kernel_spmd(nc, in_maps, list(range(N_CORES)))
    return gather_outputs(res.results, B)
